# revision 26
# baseline (speedup 1.0000x reference)
"""Trainium2 Bass kernel for MemoryEfficientMultiHeadAttention (8 NeuronCores).

Sharding: hybrid data/tensor parallel. Core c handles batch b = c//2 and head
group half = c%2 (8 of 16 heads, i.e. 512 of 1024 qkv features). Each core:
  q,k  = (x_b @ w.T + b) in [feat, tok] layout (feat on partitions)
  vT   = (x_b @ wv.T + b) in [tok, feat] layout, with a constant ones column
         appended per head (65 features/head)
  per head-pair p, query block qt (512 tokens):
    scoresT = k_h.T @ q_h  for both heads (transposed scores, [kt, qt])
    PT = exp(scoresT / 8)                (no max-subtraction: scores are O(1))
    psH[0:65] += vT1_h.T @ PT_h          (accumulate over kt tiles; row 64 is
                                          the softmax denominator via the ones
                                          column -- no separate M=1 matmuls)
  per block: r = 1/denom (DVE reciprocal straight off the PSUM row), then the
  normalization (selector-matmul broadcast + multiply) is deferred into the
  NEXT block's kt loop.  All q/k projections run as a pure-PE prologue and
  the dense partials as a pure-PE epilogue, keeping the attention loop free
  of matmul bursts (the softmax exp on the ACT engine, ~1.1us per [128,1024]
  tile, is its rate limiter; measured on HW, interleaving filler into the
  loop vs phase separation differ by <1%).
Host: out[b] = outp[2b] + outp[2b+1] + dense_b.

All matmuls run in bf16 with fp32 PSUM accumulation.  The per-head score
matmuls sit at base partitions 0/64 so they row-tile into the PE array and
run concurrently (measured 115ns/MM vs 273ns serial on this hardware).
"""

import sys
import time
from contextlib import ExitStack

import numpy as np

try:
    import concourse.bass as bass  # noqa: F401
except ImportError:  # pragma: no cover
    sys.path.insert(0, "/opt/trn_rl_repo")

import ml_dtypes

import concourse.bacc as bacc
import concourse.mybir as mybir
import concourse.tile as tile

P = 128
BF16 = mybir.dt.bfloat16
F32 = mybir.dt.float32
NPBF16 = ml_dtypes.bfloat16

B, S, D = 4, 2048, 1024
HHALF = 512  # features per core (8 heads x 64)
DV = 65  # V features per head incl. ones column

# head-selector for the reciprocal broadcast matmul: row0 -> head A feature
# partitions, row1 -> head B feature partitions
_SEL2 = np.zeros((2, P), NPBF16)
_SEL2[0, 0:64] = 1
_SEL2[1, 64:128] = 1


def _build_nc(loop_r=None):
    nc = bacc.Bacc()

    xT = nc.dram_tensor("xT", [D, S], BF16, kind="ExternalInput")
    wqT = nc.dram_tensor("wqT", [D, HHALF], BF16, kind="ExternalInput")
    wkT = nc.dram_tensor("wkT", [D, HHALF], BF16, kind="ExternalInput")
    wvT = nc.dram_tensor("wvT", [D, HHALF], BF16, kind="ExternalInput")
    dwT = nc.dram_tensor("dwT", [HHALF, D], BF16, kind="ExternalInput")
    qb = nc.dram_tensor("qb", [P, 4], F32, kind="ExternalInput")
    kb = nc.dram_tensor("kb", [P, 4], F32, kind="ExternalInput")
    vb = nc.dram_tensor("vb", [P, HHALF], BF16, kind="ExternalInput")
    sel = nc.dram_tensor("sel", [2, P], BF16, kind="ExternalInput")
    nonce = nc.dram_tensor("nonce", [1, 1], F32, kind="ExternalInput")
    outp = nc.dram_tensor("outp", [S, D], F32, kind="ExternalOutput")

    Exp = mybir.ActivationFunctionType.Exp

    with tile.TileContext(nc) as tc, ExitStack() as ctx:
        wpool = ctx.enter_context(tc.tile_pool(name="weights", bufs=1))
        spool = ctx.enter_context(tc.tile_pool(name="state", bufs=1))
        ptpool = ctx.enter_context(tc.tile_pool(name="pt", bufs=4))
        evpool = ctx.enter_context(tc.tile_pool(name="evac", bufs=4))
        rbpool = ctx.enter_context(tc.tile_pool(name="rb", bufs=10))
        ps_sc = ctx.enter_context(tc.tile_pool(name="pssc", bufs=2, space="PSUM"))
        ps_att = ctx.enter_context(tc.tile_pool(name="psatt", bufs=2, space="PSUM"))
        ps_gen = ctx.enter_context(tc.tile_pool(name="psgen", bufs=2, space="PSUM"))

        # ---- persistent SBUF state (loaded once) ----
        # load order matters: first qkproj needs wq/wk + xT tokens 0:512, so
        # queue those first and stream the rest behind them.
        xT_sb = wpool.tile([P, 8, S], BF16)
        wqT_sb = wpool.tile([P, 8, HHALF], BF16)
        wkT_sb = wpool.tile([P, 8, HHALF], BF16)
        wvT_sb = wpool.tile([P, 8, HHALF], BF16)
        dwT_sb = wpool.tile([P, 4, D], BF16)
        qb_sb = wpool.tile([P, 4], F32)
        kb_sb = wpool.tile([P, 4], F32)
        vb_sb = wpool.tile([P, HHALF], BF16)
        selA = wpool.tile([1, P], BF16)
        selB = wpool.tile([1, P], BF16)
        nonce_sb = wpool.tile([1, 1], F32)

        def load_inputs():
            xT_r = xT.rearrange("(o p) t -> p o t", p=P)
            wqT_r = wqT.rearrange("(o p) f -> p o f", p=P)
            wkT_r = wkT.rearrange("(o p) f -> p o f", p=P)
            # first q-projection (pair 0, tokens 0:512, kk 0..7) gated only
            # on the pair-0 slice of wq and the first token chunk of x
            nc.sync.dma_start(wqT_sb[:, :, 0:128], wqT_r[:, :, 0:128])
            nc.sync.dma_start(xT_sb[:, 0:4, 0:512], xT_r[:, 0:4, 0:512])
            nc.sync.dma_start(xT_sb[:, 4:8, 0:512], xT_r[:, 4:8, 0:512])
            nc.sync.dma_start(qb_sb[:], qb[:])
            nc.sync.dma_start(kb_sb[:], kb[:])
            nc.sync.dma_start(wkT_sb[:, :, 0:128], wkT_r[:, :, 0:128])
            nc.sync.dma_start(vb_sb[:], vb[:])
            nc.sync.dma_start(selA[:], sel[0:1, :])
            nc.sync.dma_start(selB[:], sel[1:2, :])
            nc.sync.dma_start(nonce_sb[:], nonce[:])
            for tc4 in range(1, 4):
                ts4 = slice(tc4 * 512, (tc4 + 1) * 512)
                nc.sync.dma_start(xT_sb[:, :, ts4], xT_r[:, :, ts4])
            nc.sync.dma_start(wqT_sb[:, :, 128:512], wqT_r[:, :, 128:512])
            nc.sync.dma_start(wkT_sb[:, :, 128:512], wkT_r[:, :, 128:512])
            nc.sync.dma_start(wvT_sb[:], wvT.rearrange("(o p) f -> p o f", p=P))
            nc.sync.dma_start(dwT_sb[:], dwT.rearrange("(o p) f -> p o f", p=P))

        q_sb = spool.tile([P, 4, S], BF16)
        k_sb = spool.tile([P, 4, S], BF16)
        vT_sb = spool.tile([P, 16, 8, DV], BF16)
        attU_sb = spool.tile([P, 4, S], BF16)
        # constant ones column per head (disjoint from v_proj writes)
        nc.vector.memset(vT_sb[:, :, :, 64:65], 1.0)

        def v_proj(t):
            ps = ps_gen.tile([P, 512], F32, tag="gen")
            for kk in range(8):
                nc.tensor.matmul(
                    ps[:],
                    lhsT=xT_sb[:, kk, t * 128 : (t + 1) * 128],
                    rhs=wvT_sb[:, kk, :],
                    start=(kk == 0),
                    stop=(kk == 7),
                )
            nc.vector.tensor_add(
                vT_sb[:, t, :, 0:64],
                ps[:].rearrange("p (h d) -> p h d", h=8),
                vb_sb[:].rearrange("p (h d) -> p h d", h=8),
            )

        def qk_group(p, t4, which):
            tok = slice(t4 * 512, (t4 + 1) * 512)
            w_sb, b_sb, dst = (
                (wqT_sb, qb_sb, q_sb) if which == "q" else (wkT_sb, kb_sb, k_sb)
            )
            ps = ps_gen.tile([P, 512], F32, tag="gen")
            for kk in range(8):
                nc.tensor.matmul(
                    ps[:],
                    lhsT=w_sb[:, kk, p * 128 : (p + 1) * 128],
                    rhs=xT_sb[:, kk, tok],
                    start=(kk == 0),
                    stop=(kk == 7),
                )
            nc.vector.tensor_scalar_add(dst[:, p, tok], ps[:], b_sb[:, p : p + 1])

        def norm_block(pp, qq, rbA, rbB):
            # broadcast 1/denom across feature partitions and normalize
            qt = slice(qq * 512, (qq + 1) * 512)
            ps_n = ps_gen.tile([P, 512], F32, tag="gen")
            nc.tensor.matmul(
                ps_n[:], lhsT=selA[0:1, :], rhs=rbA[:], start=True, stop=False
            )
            nc.tensor.matmul(
                ps_n[:], lhsT=selB[0:1, :], rhs=rbB[:], start=False, stop=True
            )
            nc.vector.tensor_mul(attU_sb[:, pp, qt], attU_sb[:, pp, qt], ps_n[:])

        def dense_group(tt, oc):
            tts = slice(tt * 128, (tt + 1) * 128)
            ocs = slice(oc * 512, (oc + 1) * 512)
            ps = ps_gen.tile([P, 512], F32, tag="gen")
            for kk in range(4):
                nc.tensor.matmul(
                    ps[:],
                    lhsT=attU_sb[:, kk, tts],
                    rhs=dwT_sb[:, kk, ocs],
                    start=(kk == 0),
                    stop=(kk == 3),
                )
            ot = evpool.tile([P, 512], F32, tag="out")
            nc.vector.tensor_copy(ot[:], ps[:])
            nc.sync.dma_start(outp[tts, ocs], ot[:])

        load_inputs()

        def body():
            pending = []
            # all q/k projections up front: a pure-PE prologue keeps the
            # attention loop free of long matmul bursts, so the ACT engine
            # (softmax exp, the rate limiter) streams without bubbles.
            for t4 in range(4):
                for p in range(4):
                    qk_group(p, t4, "q")
                    qk_group(p, t4, "k")

            for p in range(4):
                # ---- attention blocks for this pair ----
                for qtc in range(4):
                    qt = slice(qtc * 512, (qtc + 1) * 512)
                    ps_a = ps_att.tile([P, 512], F32, tag="att")
                    ps_b = ps_att.tile([P, 512], F32, tag="att")
                    prev = None
                    for kt in range(17):
                        if kt < 16:
                            kts = slice(kt * 128, (kt + 1) * 128)
                            if p == 0 and qtc == 0:
                                v_proj(kt)
                            sc = ps_sc.tile([P, 1024], F32, tag="sc")
                            # transposed scores for both heads of the pair
                            nc.tensor.matmul(
                                sc[:, 0:512],
                                lhsT=k_sb[0:64, p, kts],
                                rhs=q_sb[0:64, p, qt],
                                start=True,
                                stop=True,
                            )
                            nc.tensor.matmul(
                                sc[:, 512:1024],
                                lhsT=k_sb[64:128, p, kts],
                                rhs=q_sb[64:128, p, qt],
                                start=True,
                                stop=True,
                            )
                            pt = ptpool.tile([P, 1024], BF16, tag="pt")
                            nc.scalar.activation(pt[:], sc[:], Exp, scale=0.125)
                        if prev is not None:
                            pkt, ppt = prev
                            # attended + denominator (ones column) per head
                            nc.tensor.matmul(
                                ps_a[0:DV, :],
                                lhsT=vT_sb[:, pkt, 2 * p, :],
                                rhs=ppt[:, 0:512],
                                start=(pkt == 0),
                                stop=(pkt == 15),
                            )
                            nc.tensor.matmul(
                                ps_b[0:DV, :],
                                lhsT=vT_sb[:, pkt, 2 * p + 1, :],
                                rhs=ppt[:, 512:1024],
                                start=(pkt == 0),
                                stop=(pkt == 15),
                            )
                        if kt < 16:
                            prev = (kt, pt)
                    # ---- evacuate + stage reciprocals; normalization is
                    # deferred into the next block's kt loop as PE filler
                    rbA = rbpool.tile([1, 512], BF16, tag="rb")
                    rbB = rbpool.tile([1, 512], BF16, tag="rb")
                    with nc.allow_low_precision(reason="softmax denom recip bf16"):
                        nc.vector.reciprocal(rbA[:], ps_a[64:65, :])
                        nc.vector.reciprocal(rbB[:], ps_b[64:65, :])
                    nc.vector.tensor_copy(attU_sb[0:64, p, qt], ps_a[0:64, :])
                    nc.vector.tensor_copy(attU_sb[64:128, p, qt], ps_b[0:64, :])
                    pending.append((p, qtc, rbA, rbB))

                # flush this pair's norms between kt loops -- the attention
                # loop itself stays free of extra instructions (measured: the
                # bare scores->exp->attV chain runs at the ACT exp rate)
                while pending:
                    norm_block(*pending.pop(0))

            # dense epilogue
            for tt in range(16):
                for oc in range(2):
                    dense_group(tt, oc)

        if loop_r:
            with tc.For_i(0, loop_r, 1):
                body()
        else:
            body()

    nc.compile()
    return nc


# ---------------------------------------------------------------------------
# PJRT runner (modeled on concourse.bass2jax.run_bass_via_pjrt, but caches the
# jitted executable so repeated calls don't retrace/recompile).
# ---------------------------------------------------------------------------
_CACHE = {}


def _make_runner(loop_r=None):
    import jax
    from jax.sharding import Mesh, PartitionSpec
    from jax.experimental.shard_map import shard_map

    from concourse import bass2jax
    from concourse import mybir as _mybir

    nc = _build_nc(loop_r=loop_r)
    bass2jax.install_neuronx_cc_hook()

    partition_name = nc.partition_id_tensor.name if nc.partition_id_tensor else None
    in_names, out_names, out_avals = [], [], []
    for alloc in nc.m.functions[0].allocations:
        if not isinstance(alloc, _mybir.MemoryLocationSet):
            continue
        name = alloc.memorylocations[0].name
        if alloc.kind == "ExternalInput":
            if name != partition_name:
                in_names.append(name)
        elif alloc.kind == "ExternalOutput":
            out_names.append(name)
            out_avals.append(
                jax.core.ShapedArray(
                    tuple(alloc.tensor_shape), _mybir.dt.np(alloc.dtype)
                )
            )
    n_params = len(in_names)
    all_in_names = list(in_names) + list(out_names)
    if partition_name is not None:
        all_in_names.append(partition_name)

    def _body(*args):
        operands = list(args)
        if partition_name is not None:
            operands.append(bass2jax.partition_id_tensor())
        outs = bass2jax._bass_exec_p.bind(
            *operands,
            out_avals=tuple(out_avals),
            in_names=tuple(all_in_names),
            out_names=tuple(out_names),
            lowering_input_output_aliases=(),
            sim_require_finite=True,
            sim_require_nnan=True,
            nc=nc,
        )
        return tuple(outs)

    devices = jax.devices()[:8]
    mesh = Mesh(np.asarray(devices), ("core",))
    in_specs = (PartitionSpec("core"),) * (n_params + len(out_names))
    out_specs = (PartitionSpec("core"),) * len(out_names)
    jitted = jax.jit(
        shard_map(
            _body, mesh=mesh, in_specs=in_specs, out_specs=out_specs, check_rep=False
        ),
        keep_unused=True,
    )
    zeros = [np.zeros((8 * av.shape[0], *av.shape[1:]), av.dtype) for av in out_avals]
    return (jitted, in_names, out_names, out_avals, zeros, mesh)


def _get_runner(loop_r=None):
    key = ("runner", loop_r)
    if key not in _CACHE:
        _CACHE[key] = _make_runner(loop_r)
    return _CACHE[key]


def _prep_core_inputs(x, wq_w, wq_b, wk_w, wk_b, wv_w, wv_b, dense_w):
    """Per-core host-side shard prep. Returns list of dicts (8 cores)."""
    maps = []
    for c in range(8):
        b, half = c // 2, c % 2
        f0 = half * HHALF
        fs = slice(f0, f0 + HHALF)
        maps.append(
            {
                "xT": np.ascontiguousarray(x[b].T).astype(NPBF16),
                "wqT": np.ascontiguousarray(wq_w[fs].T).astype(NPBF16),
                "wkT": np.ascontiguousarray(wk_w[fs].T).astype(NPBF16),
                "wvT": np.ascontiguousarray(wv_w[fs].T).astype(NPBF16),
                "dwT": np.ascontiguousarray(dense_w[:, fs].T).astype(NPBF16),
                "qb": np.ascontiguousarray(wq_b[fs].reshape(4, P).T.astype(np.float32)),
                "kb": np.ascontiguousarray(wk_b[fs].reshape(4, P).T.astype(np.float32)),
                "vb": np.broadcast_to(
                    wv_b[fs].reshape(1, HHALF).astype(NPBF16), (P, HHALF)
                ).copy(),
                "sel": _SEL2,
                "nonce": np.zeros((1, 1), np.float32),
            }
        )
    return maps


def run_device(in_maps, time_iters=0, loop_r=None):
    """Run the SPMD kernel. Returns (per-core outp list, best wall ns or None)."""
    jitted, in_names, out_names, out_avals, zeros, mesh = _get_runner(loop_r)
    concat_in = [
        np.concatenate([in_maps[c][name] for c in range(8)], axis=0)
        for name in in_names
    ]
    args = concat_in + zeros
    outs = jitted(*args)
    outs = [np.asarray(o) for o in outs]
    best_ns = None
    if time_iters:
        import jax
        from jax.sharding import NamedSharding, PartitionSpec

        sh = NamedSharding(mesh, PartitionSpec("core"))
        dev_args = [jax.device_put(a, sh) for a in args]
        jax.block_until_ready(dev_args)
        times = []
        for _ in range(time_iters):
            t0 = time.perf_counter()
            o = jitted(*dev_args)
            jax.block_until_ready(o)
            times.append(time.perf_counter() - t0)
        best_ns = int(min(times) * 1e9)
    per_core = [
        {
            name: outs[i].reshape(8, *out_avals[i].shape)[c]
            for i, name in enumerate(out_names)
        }
        for c in range(8)
    ]
    return per_core, best_ns


def kernel(**inputs):
    x = np.asarray(inputs["x"], np.float32)
    args = {
        k: np.asarray(inputs[k], np.float32)
        for k in ["wq_w", "wq_b", "wk_w", "wk_b", "wv_w", "wv_b", "dense_w"]
    }
    in_maps = _prep_core_inputs(x, **args)
    per_core, _ = run_device(in_maps)
    dense_b = np.asarray(inputs["dense_b"], np.float32)
    out = np.empty((B, S, D), np.float32)
    for b in range(B):
        out[b] = per_core[2 * b]["outp"] + per_core[2 * b + 1]["outp"] + dense_b
    return out
